# revision 11
# baseline (speedup 1.0000x reference)
"""Trainium2 Bass kernel for nn_PixelEachSubstitutor (8-core data parallel).

Math (validated against the jax reference by a numpy golden model):
  - Only the 9 window tokens of the 49-token canvas are ever nonzero; the
    key-padding mask is constant and masked tokens never feed back into real
    tokens -> encoder P runs with sequence length 9.
  - Every encoder has head_dim == 1, so attention is per-head scalar outer
    products + softmax over 9/10/49 keys.  Score magnitudes are bounded by
    LN, so softmax runs without max-subtraction (validated: 4.5e-6 rel err).
  - Device layout: partitions = (batch-subgroup bt, feature/head), free =
    (batch-chunk b2, token).  All matmuls use block-diagonal stationary
    weights; LN mean/var via block-diag centering matmuls on the PE.

Per core: 225 sequences.  encP: b = bt*21+b2 (11x21, pad 231);
encL: b = bt*114+b2 (2x114, pad 228); encC: b = bt*19+b2 (12x19, pad 228).
Layout transitions are routed through DRAM scratch with affine DMAs.
"""
import os
import sys

for _p in ("/opt/trn_rl_repo", os.path.expanduser("~/.axon_site/_ro/trn_rl_repo")):
    if os.path.isdir(_p) and _p not in sys.path:
        sys.path.insert(0, _p)

import numpy as np

NUM_CLASSES = 10
D_PAD = 11
L = 49
EPS = 1e-5
BC = 225
P_BT, P_B2 = 11, 21      # encP: 121 partitions, F=189
L_BT, L_B2 = 2, 114      # encL: 98  partitions, F=1140
C_BT, C_B2 = 12, 19      # encC: 120 partitions, F=931
F_P = P_B2 * 9           # 189
F_L = L_B2 * 10          # 1140
F_C = C_B2 * L           # 931
REAL9 = [0, 1, 2, 7, 8, 9, 14, 15, 16]

CHUNKS_P = [(0, 189)]
CHUNKS_L = [(0, 380), (380, 380), (760, 380)]
CHUNKS_C = [(0, 466), (466, 465)]


# --------------------------------------------------------------------------
# host-side input staging (pure layout/gather, no model arithmetic)
# --------------------------------------------------------------------------

def build_x0(x_full, core):
    N, C, H, W = x_full.shape
    xp = np.zeros((N, C + 1, H + 2, W + 2), np.float32)
    xp[:, :C, 1:H + 1, 1:W + 1] = x_full
    xp[:, C, :, :] = 1.0
    xp[:, C, 1:H + 1, 1:W + 1] = 0.0
    x0 = np.zeros((121, F_P), np.float32)
    for bl in range(BC):
        bg = BC * core + bl
        n, i, j = np.unravel_index(bg, (N, H, W))
        bt, b2 = bl // P_B2, bl % P_B2
        for t in range(9):
            di, dj = t // 3, t % 3
            x0[bt * 11:bt * 11 + 11, b2 * 9 + t] = xp[n, :, i + di, j + dj]
    return x0


def assemble_output(y_cores):
    out = np.zeros((2, NUM_CLASSES, 30, 30), np.float32)
    for core, y in enumerate(y_cores):
        for bl in range(BC):
            bg = BC * core + bl
            n, i, j = np.unravel_index(bg, (2, 30, 30))
            bt, b2 = bl // C_B2, bl % C_B2
            out[n, :, i, j] = y[bt * 10:bt * 10 + 10, b2]
    return out


# --------------------------------------------------------------------------
# weight packing (host builds the values; device uses the same offsets)
# --------------------------------------------------------------------------

def center(d):
    return np.eye(d, dtype=np.float32) - np.full((d, d), 1.0 / d, np.float32)


def bd(A, n):
    return np.kron(np.eye(n, dtype=np.float32), A.astype(np.float32))


class Pack:
    """Allocates [K, M] matrices as column ranges of a [128, N] array."""

    def __init__(self):
        self.off = {}
        self.n = 0
        self.mats = []

    def add(self, name, mat):
        K, M = mat.shape
        assert K <= 128
        self.off[name] = (self.n, K, M)
        self.mats.append(mat.astype(np.float32))
        self.n += M

    def array(self):
        a = np.zeros((128, self.n), np.float32)
        for (c0, K, M), m in zip(self.off.values(), self.mats):
            a[:K, c0:c0 + M] = m
        return a


def build_packs(W):
    """W: raw reference weights. Returns (wpack Pack, vecs [128, NV])."""
    pk = Pack()
    # ---- encP ----
    C11 = center(D_PAD)
    for l in range(6):
        Wq, Wk, Wv = W['pWin'][l][:11], W['pWin'][l][11:22], W['pWin'][l][22:]
        pk.add(f"Pq{l}", bd(Wq.T, P_BT))
        pk.add(f"Pk{l}", bd(Wk.T, P_BT))
        pk.add(f"Pv{l}", bd(Wv.T, P_BT))
        pk.add(f"Pwo{l}", bd((C11 @ W['pWout'][l]).T, P_BT))
        # FFN1: full-K shifted blockdiag, groups of bt: [0..3], [4..7], [8..10]
        for m, grp in enumerate(([0, 1, 2, 3], [4, 5, 6, 7], [8, 9, 10])):
            f1 = np.zeros((121, 32 * len(grp)), np.float32)
            f2 = np.zeros((32 * len(grp), 121), np.float32)
            cf2 = C11 @ W['pWf2'][l]
            for gi, bt in enumerate(grp):
                f1[bt * 11:bt * 11 + 11, gi * 32:(gi + 1) * 32] = W['pWf1'][l].T
                f2[gi * 32:(gi + 1) * 32, bt * 11:bt * 11 + 11] = cf2.T
            pk.add(f"Pf1{l}_{m}", f1)
            pk.add(f"Pf2{l}_{m}", f2)
    pk.add("PC", bd(C11, P_BT))
    pk.add("Pones", bd(np.ones((11, 1), np.float32), P_BT))
    pk.add("Pbc", bd(np.ones((1, 11), np.float32), P_BT))
    sel_s = np.zeros((121, 11), np.float32)
    sel_p = np.zeros((121, 11), np.float32)
    for bt in range(P_BT):
        sel_s[bt * 11:bt * 11 + 10, bt] = 1.0
        sel_p[bt * 11 + 10, bt] = 1.0
    pk.add("PselS", sel_s)
    pk.add("PselP", sel_p)
    # ---- encL ----
    C49 = center(L)
    for l in range(6):
        Wq, Wk, Wv = W['LWin'][l][:49], W['LWin'][l][49:98], W['LWin'][l][98:]
        pk.add(f"Lq{l}", bd(Wq.T, L_BT))
        pk.add(f"Lk{l}", bd(Wk.T, L_BT))
        pk.add(f"Lv{l}", bd(Wv.T, L_BT))
        pk.add(f"Lwo{l}", bd((C49 @ W['LWout'][l]).T, L_BT))
        pk.add(f"Lf1{l}", bd(W['LWf1'][l].T, L_BT))
        pk.add(f"Lf2{l}", bd((C49 @ W['LWf2'][l]).T, L_BT))
    pk.add("LC", bd(C49, L_BT))
    pk.add("Lones", bd(np.ones((49, 1), np.float32), L_BT))
    pk.add("Lbc", bd(np.ones((1, 49), np.float32), L_BT))
    # ---- encC ----
    C10 = center(NUM_CLASSES)
    Wq, Wk, Wv = W['CWin'][0][:10], W['CWin'][0][10:20], W['CWin'][0][20:]
    pk.add("Cq0", bd(Wq.T, C_BT))
    pk.add("Ck0", bd(Wk.T, C_BT))
    pk.add("Cv0", bd(Wv.T, C_BT))
    pk.add("Cwo0", bd((C10 @ W['CWout'][0]).T, C_BT))
    pk.add("Cf10", bd(W['CWf1'][0].T, C_BT))
    pk.add("Cf20", bd((C10 @ W['CWf2'][0]).T, C_BT))
    pk.add("CC", bd(C10, C_BT))
    pk.add("Cones", bd(np.ones((10, 1), np.float32), C_BT))
    pk.add("Cbc", bd(np.ones((1, 10), np.float32), C_BT))
    pk.add("wdrep", np.tile(W['Wdec'][0][None, :], (120, 1)))
    # ---- vecs: per-partition LN scale columns ----
    NV = 27
    vecs = np.zeros((128, NV), np.float32)
    vecs[:, 26] = EPS
    for l in range(6):
        vecs[:121, 2 * l] = np.tile(W['pln1'][l], P_BT)
        vecs[:121, 2 * l + 1] = np.tile(W['pln2'][l], P_BT)
        vecs[:98, 12 + 2 * l] = np.tile(W['Lln1'][l], L_BT)
        vecs[:98, 12 + 2 * l + 1] = np.tile(W['Lln2'][l], L_BT)
    vecs[:120, 24] = np.tile(W['Cln1'][0], C_BT)
    vecs[:120, 25] = np.tile(W['Cln2'][0], C_BT)
    return pk, vecs


# --------------------------------------------------------------------------
# device kernel
# --------------------------------------------------------------------------

def _patch_tail_drain(tile_mod, ScopedClock, VectorClock):
    """This walrus build can't encode one drain carrying many sem waits;
    split the TileContext tail drain into one single-wait drain per proc."""
    if getattr(tile_mod.TileContext, "_tail_patched", False):
        return

    def _drain_and_barrier(self, tick_clock, wait_clock):
        gc = tick_clock.global_clock
        n = len(gc)
        for i in range(n):
            t = gc[i]
            if t <= 0:
                continue
            vec = [0] * n
            vec[i] = t
            d = self.nc.sync.drain()
            wait_clock.add_sem_waits(d.ins, ScopedClock({None: VectorClock(vec)}))
        self.nc.sync.drain()
        self.nc.all_engine_barrier()
        assert self.sems is not None
        popped = self.nc._tile_sem_poison_stack.pop()
        assert popped is self._sem_poison
        self.nc.clear_and_free_semaphores(list(self.sems.allocated().values()))
        self.nc.all_engine_barrier()

    tile_mod.TileContext._drain_and_barrier = _drain_and_barrier
    tile_mod.TileContext._tail_patched = True


def build_bass_program():
    import concourse.bass as bass
    import concourse.mybir as mybir
    import concourse.tile as tile_mod
    import concourse.tile_sem_assignment as tsa
    from concourse.vector_clock import ScopedClock, VectorClock

    _patch_tail_drain(tile_mod, ScopedClock, VectorClock)
    # this walrus rejects instructions carrying many sem waits; keep every
    # HWDGE DMA on one proc/sem (all DMAs issue on the SP ring -> FIFO-safe)
    tsa.NUM_HWDGE_SEMS = 1

    f32 = mybir.dt.float32
    AF = mybir.ActivationFunctionType
    ALU = mybir.AluOpType
    AX = mybir.AxisListType

    pk, vecs_arr = _PACKS  # layout only (values already in _PACKS arrays)
    NW = pk.n

    nc = bass.Bass("TRN2", target_bir_lowering=False, debug=False, num_devices=1)
    x0_d = nc.dram_tensor("x0", [121, F_P], f32, kind="ExternalInput")
    wp_d = nc.dram_tensor("wp", [128, NW], f32, kind="ExternalInput")
    vec_d = nc.dram_tensor("vecs", [128, 27], f32, kind="ExternalInput")
    y_d = nc.dram_tensor("y", [120, C_B2], f32, kind="ExternalOutput")
    zp_d = nc.dram_tensor("zp_scr", [121 * F_P], f32, kind="Internal")
    zl_d = nc.dram_tensor("zl_scr", [98 * F_L], f32, kind="Internal")

    def APX(t, free_dims, extra_off=0):
        # t: a 2D [parts, F] tile AP; free_dims: [[step, count], ...] within a row
        pstep, pcnt = t.ap[0]
        return bass.AP(tensor=t.tensor, offset=t.offset + extra_off,
                       ap=[[pstep, pcnt]] + free_dims)

    with tile_mod.TileContext(nc) as tc:
        ctx_pools = {}

        def wap(name):
            c0, K, M = pk.off[name]
            return wtile[0:K, c0:c0 + M]

        with tc.tile_pool(name="persist", bufs=1) as persist:
            wtile = persist.tile([128, NW], f32)
            nc.sync.dma_start(wtile[:], wp_d[:])
            vtile = persist.tile([128, 27], f32)
            nc.sync.dma_start(vtile[:], vec_d[:])

            # ---------------- generic building blocks ----------------
            def mm_to_sbuf(psum, terms, out_sb, parts, chunks, func=AF.Copy,
                           tag="mm"):
                """psum-accumulated matmul terms, then ACT func -> out_sb."""
                for (c0, cn) in chunks:
                    ps = psum.tile([parts, cn], f32, tag=tag)
                    n = len(terms)
                    for i, (lhsT, rhs) in enumerate(terms):
                        nc.tensor.matmul(ps[:, :], lhsT, rhs[:, c0:c0 + cn],
                                         start=(i == 0), stop=(i == n - 1))
                    if func == AF.Copy:
                        nc.scalar.copy(out_sb[:, c0:c0 + cn], ps[:, :])
                    else:
                        nc.scalar.activation(out_sb[:, c0:c0 + cn], ps[:, :], func)

            def layer_norm(psum, sb, terms, parts, bt, d, F, chunks, w_ap,
                           Cm, ones, bc, out_sb):
                xc = sb.tile([parts, F], f32, tag="xc")
                sq = sb.tile([parts, F], f32, tag="sq")
                sd = sb.tile([bt, F], f32, tag="sd")
                for (c0, cn) in chunks:
                    ps = psum.tile([parts, cn], f32, tag="mm")
                    allt = [(Cm, terms[0][1])] + [(m, r) for (m, r) in terms[1:]]
                    n = len(allt)
                    for i, (lhsT, rhs) in enumerate(allt):
                        nc.tensor.matmul(ps[:, :], lhsT, rhs[:, c0:c0 + cn],
                                         start=(i == 0), stop=(i == n - 1))
                    nc.scalar.copy(xc[:, c0:c0 + cn], ps[:, :])
                    nc.scalar.activation(sq[:, c0:c0 + cn], xc[:, c0:c0 + cn],
                                         AF.Square)
                    vps = psum.tile([bt, cn], f32, tag="mmv")
                    nc.tensor.matmul(vps[:, :], ones, sq[:, c0:c0 + cn])
                    nc.scalar.activation(sd[:, c0:c0 + cn], vps[:, :], AF.Sqrt,
                                         bias=vtile[0:bt, 26:27], scale=1.0 / d)
                nc.vector.reciprocal(sd[:, :], sd[:, :])
                for (c0, cn) in chunks:
                    bps = psum.tile([parts, cn], f32, tag="mm")
                    nc.tensor.matmul(bps[:, :], bc, sd[:, c0:c0 + cn])
                    nc.vector.scalar_tensor_tensor(
                        out=out_sb[:, c0:c0 + cn], in0=xc[:, c0:c0 + cn],
                        scalar=w_ap, in1=bps[:, :],
                        op0=ALU.mult, op1=ALU.mult)

            def attention(psum, sb, spool, x, lq, lk, lv, parts, b2, ntok, F,
                          chunks, o_out):
                q = sb.tile([parts, F], f32, tag="q")
                k = sb.tile([parts, F], f32, tag="k")
                v = sb.tile([parts, F], f32, tag="v")
                mm_to_sbuf(psum, [(lq, x)], q, parts, chunks)
                mm_to_sbuf(psum, [(lk, x)], k, parts, chunks)
                mm_to_sbuf(psum, [(lv, x)], v, parts, chunks)
                den = sb.tile([parts, F], f32, tag="den")
                num = sb.tile([parts, F], f32, tag="num")
                # chunk over b2 so the score tile stays bounded
                bstep = max(1, min(b2, 6000 // (ntok * ntok)))
                for b0 in range(0, b2, bstep):
                    bn = min(bstep, b2 - b0)
                    s = spool.tile([parts, bstep * ntok * ntok], f32, tag="s")
                    q4 = APX(q, [[ntok, bn], [1, ntok], [0, ntok]], b0 * ntok)
                    k4 = APX(k, [[ntok, bn], [0, ntok], [1, ntok]], b0 * ntok)
                    v4 = APX(v, [[ntok, bn], [0, ntok], [1, ntok]], b0 * ntok)
                    s4 = APX(s, [[ntok * ntok, bn], [ntok, ntok], [1, ntok]])
                    s3 = APX(s, [[ntok, bn * ntok], [1, ntok]])
                    nc.vector.tensor_mul(s4, q4, k4)
                    nc.scalar.activation(s[:, :], s[:, :], AF.Exp)
                    nc.vector.tensor_reduce(
                        den[:, b0 * ntok:(b0 + bn) * ntok], s3,
                        axis=AX.X, op=ALU.add)
                    nc.vector.tensor_mul(s4, s4, v4)
                    nc.vector.tensor_reduce(
                        num[:, b0 * ntok:(b0 + bn) * ntok], s3,
                        axis=AX.X, op=ALU.add)
                nc.vector.reciprocal(den[:, :], den[:, :])
                nc.vector.tensor_mul(o_out[:, :], num[:, :], den[:, :])

            def enc_layer(psum, sb, spool, x_sb, pre, l, parts, bt, d, b2,
                          ntok, F, chunks, hid_terms, w1_ap, w2_ap, Cm, ones,
                          bc):
                o = sb.tile([parts, F], f32, tag="o")
                attention(psum, sb, spool, x_sb, wap(f"{pre}q{l}"),
                          wap(f"{pre}k{l}"), wap(f"{pre}v{l}"), parts, b2,
                          ntok, F, chunks, o)
                x1 = sb.tile([parts, F], f32, tag="x1")
                layer_norm(psum, sb, [(None, x_sb), (wap(f"{pre}wo{l}"), o)],
                           parts, bt, d, F, chunks, w1_ap, Cm, ones, bc, x1)
                # FFN
                f2_terms = []
                for (f1name, f2name, hparts) in hid_terms(l):
                    h = sb.tile([hparts, F], f32, tag=f"h{f1name[-1]}")
                    mm_to_sbuf(psum, [(wap(f1name), x1)], h, hparts, chunks,
                               func=AF.Relu)
                    f2_terms.append((wap(f2name), h))
                x2 = sb.tile([parts, F], f32, tag="x2")
                layer_norm(psum, sb, [(None, x1)] + f2_terms,
                           parts, bt, d, F, chunks, w2_ap, Cm, ones, bc, x2)
                return x2

            # ---------------- stage A: encP ----------------
            with tc.tile_pool(name="sbP", bufs=1) as sbP, \
                 tc.tile_pool(name="ssP", bufs=2) as ssP, \
                 tc.tile_pool(name="psP", bufs=2, space="PSUM") as psP:
                x = sbP.tile([121, F_P], f32, tag="x0")
                nc.sync.dma_start(x[:], x0_d[:])
                x0_keep = x

                def hidP(l):
                    return [(f"Pf1{l}_0", f"Pf2{l}_0", 128),
                            (f"Pf1{l}_1", f"Pf2{l}_1", 128),
                            (f"Pf1{l}_2", f"Pf2{l}_2", 96)]

                for l in range(6):
                    x = enc_layer(psP, sbP, ssP, x, "P", l, 121, P_BT, D_PAD,
                                  P_B2, 9, F_P, CHUNKS_P, hidP,
                                  vtile[0:121, 2 * l:2 * l + 1],
                                  vtile[0:121, 2 * l + 1:2 * l + 2],
                                  wap("PC"), wap("Pones"), wap("Pbc"))

                # cp + z build
                eh = sbP.tile([121, F_P], f32, tag="eh")
                nc.scalar.activation(eh[:, :], x[:, :], AF.Exp)
                dps = psP.tile([11, F_P], f32, tag="mmv")
                nc.tensor.matmul(dps[:, :], wap("PselS"), eh[:, :])
                mps = psP.tile([11, F_P], f32, tag="mmv2")
                nc.tensor.matmul(mps[:, :], wap("PselP"), x0_keep[:, :])
                denr = sbP.tile([11, F_P], f32, tag="denr")
                nc.vector.reciprocal(denr[:, :], dps[:, :])
                scl = sbP.tile([11, F_P], f32, tag="scl")
                nc.vector.tensor_mul(scl[:, :], denr[:, :], mps[:, :])
                sps = psP.tile([121, F_P], f32, tag="mm")
                nc.tensor.matmul(sps[:, :], wap("Pbc"), scl[:, :])
                zp = sbP.tile([121, F_P], f32, tag="zp")
                nc.vector.tensor_mul(zp[:, :], eh[:, :], sps[:, :])
                nc.vector.tensor_add(zp[:, :], zp[:, :], x0_keep[:, :])
                for bt in range(P_BT):
                    src = zp[bt * 11:(bt + 1) * 11, :].rearrange(
                        "c (b t) -> c b t", t=9)
                    dst = bass.AP(tensor=zp_d, offset=bt * 21 * 99,
                                  ap=[[1, 11], [99, 21], [11, 9]])
                    nc.sync.dma_start(dst, src)

            # ---------------- A -> B remap ----------------
            with tc.tile_pool(name="sbL", bufs=1) as sbL, \
                 tc.tile_pool(name="ssL", bufs=2) as ssL, \
                 tc.tile_pool(name="psL", bufs=2, space="PSUM") as psL:
                zl = sbL.tile([98, F_L], f32, tag="x")
                nc.gpsimd.memset(zl[:, :], 0.0)
                for btL in range(L_BT):
                    for di in range(3):
                        dst = zl[btL * 49 + 7 * di:btL * 49 + 7 * di + 3, :
                                 ].rearrange("p (b c) -> p b c", c=10)
                        src = bass.AP(
                            tensor=zp_d,
                            offset=btL * L_B2 * 99 + 33 * di,
                            ap=[[11, 3], [99, L_B2], [1, 10]])
                        nc.sync.dma_start(dst, src)

                # ---------------- stage B: encL ----------------
                def hidL(l):
                    return [(f"Lf1{l}", f"Lf2{l}", 2)]

                xL = zl
                for l in range(6):
                    xL = enc_layer(psL, sbL, ssL, xL, "L", l, 98, L_BT, L,
                                   L_B2, 10, F_L, CHUNKS_L, hidL,
                                   vtile[0:98, 12 + 2 * l:12 + 2 * l + 1],
                                   vtile[0:98, 12 + 2 * l + 1:12 + 2 * l + 2],
                                   wap("LC"), wap("Lones"), wap("Lbc"))
                for btL in range(L_BT):
                    src = xL[btL * 49:(btL + 1) * 49, :].rearrange(
                        "l (b c) -> l b c", c=10)
                    dst = bass.AP(tensor=zl_d, offset=btL * L_B2 * 490,
                                  ap=[[1, 49], [490, L_B2], [49, 10]])
                    nc.sync.dma_start(dst, src)

            # ---------------- B -> C remap + stage C ----------------
            with tc.tile_pool(name="sbC", bufs=1) as sbC, \
                 tc.tile_pool(name="psC", bufs=2, space="PSUM") as psC, \
                 tc.tile_pool(name="ssC", bufs=2) as ssC:
                zc = sbC.tile([120, F_C], f32, tag="x")
                for btC in range(C_BT):
                    src = bass.AP(tensor=zl_d, offset=C_B2 * btC * 490,
                                  ap=[[49, 10], [490, C_B2], [1, 49]])
                    dst = zc[btC * 10:btC * 10 + 10, :].rearrange(
                        "p (b l) -> p b l", l=49)
                    nc.sync.dma_start(dst, src)

                def hidC(l):
                    return [("Cf10", "Cf20", 12)]

                xC = enc_layer(psC, sbC, ssC, zc, "C", 0, 120, C_BT,
                               NUM_CLASSES, C_B2, L, F_C, CHUNKS_C, hidC,
                               vtile[0:120, 24:25], vtile[0:120, 25:26],
                               wap("CC"), wap("Cones"), wap("Cbc"))

                # decode: y = sum_l xC * Wdec[l]
                wd = wap("wdrep")
                tprod = sbC.tile([120, F_C], f32, tag="tp")
                wd3 = APX(wd, [[0, C_B2], [1, 49]])
                x3 = APX(xC, [[49, C_B2], [1, 49]])
                t3 = APX(tprod, [[49, C_B2], [1, 49]])
                nc.vector.tensor_mul(t3, x3, wd3)
                ytile = sbC.tile([120, C_B2], f32, tag="y")
                nc.vector.tensor_reduce(
                    ytile[:, :], APX(tprod, [[49, C_B2], [1, 49]]),
                    axis=AX.X, op=ALU.add)
                nc.sync.dma_start(y_d[:], ytile[:, :])

    # walrus in this toolchain enforces <=1 sem wait per instruction
    # (2 for EventSemaphore); run the bacc normalization passes.
    import bass_rust as _bass_rust
    _bass_rust.move_matmul_waits_to_ldweights(nc.m)
    _bass_rust.generate_event_semaphores(nc)
    return nc


def APX_D(bass, dram_handle, off, ap):
    return bass.AP(tensor=dram_handle, offset=off, ap=ap)


_PACKS = None


def _install_ntff_hook():
    """This image's antenv lacks axon_hooks; synthesize it so trace=True
    can capture NTFF profiles via the injected libaxon_pjrt.so."""
    import types
    try:
        import antenv.axon_hooks  # noqa: F401
        return
    except ImportError:
        pass
    try:
        from trn_agent_boot.trn_boot import _ntff_profile_via_ctypes
    except ImportError:
        sys.path.insert(0, os.path.expanduser("~/.axon_site"))
        from trn_agent_boot.trn_boot import _ntff_profile_via_ctypes
    hook = None
    for so in ("/opt/axon/libaxon_pjrt.so",):
        if os.path.exists(so):
            hook = _ntff_profile_via_ctypes(so)
            break
    mod = types.ModuleType("antenv.axon_hooks")
    mod.get_axon_ntff_profile_hook = lambda: hook
    mod.set_axon_ntff_profile_hook = lambda h: None
    import antenv
    antenv.axon_hooks = mod
    sys.modules["antenv.axon_hooks"] = mod


def kernel(**inputs):
    global _PACKS
    W = {k: np.asarray(v, np.float32) for k, v in inputs.items()}
    x_full = W.pop('x')
    pk, vecs_arr = build_packs(W)
    _PACKS = (pk, vecs_arr)
    wpack_arr = pk.array()

    nc = build_bass_program()

    from concourse.bass_utils import run_bass_kernel_spmd
    trace = os.environ.get("KERNEL_TRACE", "") == "1"
    if trace:
        _install_ntff_hook()
    in_maps = []
    for core in range(8):
        in_maps.append({
            "x0": build_x0(x_full, core),
            "wp": wpack_arr,
            "vecs": vecs_arr,
        })
    res = run_bass_kernel_spmd(nc, in_maps, core_ids=list(range(8)),
                               trace=trace)
    kernel.last_result = res
    ys = [res.results[i]["y"] for i in range(8)]
    return assemble_output(ys)


if __name__ == "__main__":
    rng = np.random.default_rng(0)
    print("building program only (syntax check)...")
    # minimal fake weights for a build check
    W = {
        'pWin': rng.standard_normal((6, 33, 11)), 'pWout': rng.standard_normal((6, 11, 11)),
        'pWf1': rng.standard_normal((6, 32, 11)), 'pWf2': rng.standard_normal((6, 11, 32)),
        'pln1': np.ones((6, 11)), 'pln2': np.ones((6, 11)),
        'LWin': rng.standard_normal((6, 147, 49)), 'LWout': rng.standard_normal((6, 49, 49)),
        'LWf1': rng.standard_normal((6, 1, 49)), 'LWf2': rng.standard_normal((6, 49, 1)),
        'Lln1': np.ones((6, 49)), 'Lln2': np.ones((6, 49)),
        'CWin': rng.standard_normal((1, 30, 10)), 'CWout': rng.standard_normal((1, 10, 10)),
        'CWf1': rng.standard_normal((1, 1, 10)), 'CWf2': rng.standard_normal((1, 10, 1)),
        'Cln1': np.ones((1, 10)), 'Cln2': np.ones((1, 10)),
        'Wdec': rng.standard_normal((1, 49)),
    }
    W = {k: np.asarray(v, np.float32) for k, v in W.items()}
    pk, vecs_arr = build_packs(W)
    _PACKS = (pk, vecs_arr)
    print("wpack cols:", pk.n)
    nc = build_bass_program()
    print("program built OK")


# revision 16
# speedup vs baseline: 1.2684x; 1.2684x over previous
"""Trainium2 Bass kernel for nn_PixelEachSubstitutor (8-core data parallel).

Math (validated against the jax reference by a numpy golden model):
  - Only the 9 window tokens of the 49-token canvas are ever nonzero; the
    key-padding mask is constant and masked tokens never feed back into real
    tokens -> encoder P runs with sequence length 9.
  - Every encoder has head_dim == 1, so attention is per-head scalar outer
    products + softmax over 9/10/49 keys.  Score magnitudes are bounded by
    LN, so softmax runs without max-subtraction (validated: 4.5e-6 rel err).
  - Device layout: partitions = (batch-subgroup bt, feature/head), free =
    (batch-chunk b2, token).  All matmuls use block-diagonal stationary
    weights; LN mean/var via block-diag centering matmuls on the PE.

Per core: 225 sequences.  encP: b = bt*21+b2 (11x21, pad 231);
encL: b = bt*114+b2 (2x114, pad 228); encC: b = bt*19+b2 (12x19, pad 228).
Layout transitions are routed through DRAM scratch with affine DMAs.
"""
import os
import sys

for _p in ("/opt/trn_rl_repo", os.path.expanduser("~/.axon_site/_ro/trn_rl_repo")):
    if os.path.isdir(_p) and _p not in sys.path:
        sys.path.insert(0, _p)

import numpy as np

NUM_CLASSES = 10
D_PAD = 11
L = 49
EPS = 1e-5
BC = 225
P_BT, P_B2 = 11, 21      # encP: 121 partitions, F=189
L_BT, L_B2 = 2, 114      # encL: 98  partitions, F=1140
C_BT, C_B2 = 12, 19      # encC: 120 partitions, F=931
F_P = P_B2 * 9           # 189
F_L = L_B2 * 10          # 1140
F_C = C_B2 * L           # 931
REAL9 = [0, 1, 2, 7, 8, 9, 14, 15, 16]

CHUNKS_P = [(0, 189)]
CHUNKS_L = [(0, 380), (380, 380), (760, 380)]
CHUNKS_C = [(0, 466), (466, 465)]


# --------------------------------------------------------------------------
# host-side input staging (pure layout/gather, no model arithmetic)
# --------------------------------------------------------------------------

def build_x0(x_full, core):
    N, C, H, W = x_full.shape
    xp = np.zeros((N, C + 1, H + 2, W + 2), np.float32)
    xp[:, :C, 1:H + 1, 1:W + 1] = x_full
    xp[:, C, :, :] = 1.0
    xp[:, C, 1:H + 1, 1:W + 1] = 0.0
    x0 = np.zeros((121, F_P), np.float32)
    for bl in range(BC):
        bg = BC * core + bl
        n, i, j = np.unravel_index(bg, (N, H, W))
        bt, b2 = bl // P_B2, bl % P_B2
        for t in range(9):
            di, dj = t // 3, t % 3
            x0[bt * 11:bt * 11 + 11, b2 * 9 + t] = xp[n, :, i + di, j + dj]
    return x0


def assemble_output(y_cores):
    out = np.zeros((2, NUM_CLASSES, 30, 30), np.float32)
    for core, y in enumerate(y_cores):
        for bl in range(BC):
            bg = BC * core + bl
            n, i, j = np.unravel_index(bg, (2, 30, 30))
            bt, b2 = bl // C_B2, bl % C_B2
            out[n, :, i, j] = y[bt * 10:bt * 10 + 10, b2]
    return out


# --------------------------------------------------------------------------
# weight packing (host builds the values; device uses the same offsets)
# --------------------------------------------------------------------------

def center(d):
    return np.eye(d, dtype=np.float32) - np.full((d, d), 1.0 / d, np.float32)


def bd(A, n):
    return np.kron(np.eye(n, dtype=np.float32), A.astype(np.float32))


class Pack:
    """Allocates [K, M] matrices as column ranges of a [128, N] array."""

    def __init__(self):
        self.off = {}
        self.n = 0
        self.mats = []

    def add(self, name, mat):
        K, M = mat.shape
        assert K <= 128
        self.off[name] = (self.n, K, M)
        self.mats.append(mat.astype(np.float32))
        self.n += M

    def array(self):
        a = np.zeros((128, self.n), np.float32)
        for (c0, K, M), m in zip(self.off.values(), self.mats):
            a[:K, c0:c0 + M] = m
        return a


def build_packs(W):
    """W: raw reference weights. Returns (wpack Pack, vecs [128, NV])."""
    pk = Pack()
    # ---- encP ----
    C11 = center(D_PAD)
    for l in range(6):
        Wq, Wk, Wv = W['pWin'][l][:11], W['pWin'][l][11:22], W['pWin'][l][22:]
        pk.add(f"Pq{l}", bd(Wq.T, P_BT))
        pk.add(f"Pk{l}", bd(Wk.T, P_BT))
        pk.add(f"Pv{l}", bd(Wv.T, P_BT))
        pk.add(f"Pwo{l}", bd((C11 @ W['pWout'][l]).T, P_BT))
        # FFN1: full-K shifted blockdiag, groups of bt: [0..3], [4..7], [8..10]
        for m, grp in enumerate(([0, 1, 2, 3], [4, 5, 6, 7], [8, 9, 10])):
            f1 = np.zeros((121, 32 * len(grp)), np.float32)
            f2 = np.zeros((32 * len(grp), 121), np.float32)
            cf2 = C11 @ W['pWf2'][l]
            for gi, bt in enumerate(grp):
                f1[bt * 11:bt * 11 + 11, gi * 32:(gi + 1) * 32] = W['pWf1'][l].T
                f2[gi * 32:(gi + 1) * 32, bt * 11:bt * 11 + 11] = cf2.T
            pk.add(f"Pf1{l}_{m}", f1)
            pk.add(f"Pf2{l}_{m}", f2)
    pk.add("PC", bd(C11, P_BT))
    pk.add("Pones", bd(np.ones((11, 1), np.float32), P_BT))
    pk.add("Pbc", bd(np.ones((1, 11), np.float32), P_BT))
    sel_s = np.zeros((121, 11), np.float32)
    sel_p = np.zeros((121, 11), np.float32)
    for bt in range(P_BT):
        sel_s[bt * 11:bt * 11 + 10, bt] = 1.0
        sel_p[bt * 11 + 10, bt] = 1.0
    pk.add("PselS", sel_s)
    pk.add("PselP", sel_p)
    # ---- encL ----
    C49 = center(L)
    for l in range(6):
        Wq, Wk, Wv = W['LWin'][l][:49], W['LWin'][l][49:98], W['LWin'][l][98:]
        pk.add(f"Lq{l}", bd(Wq.T, L_BT))
        pk.add(f"Lk{l}", bd(Wk.T, L_BT))
        pk.add(f"Lv{l}", bd(Wv.T, L_BT))
        pk.add(f"Lwo{l}", bd((C49 @ W['LWout'][l]).T, L_BT))
        pk.add(f"Lf1{l}", bd(W['LWf1'][l].T, L_BT))
        pk.add(f"Lf2{l}", bd((C49 @ W['LWf2'][l]).T, L_BT))
    pk.add("LC", bd(C49, L_BT))
    pk.add("Lones", bd(np.ones((49, 1), np.float32), L_BT))
    pk.add("Lbc", bd(np.ones((1, 49), np.float32), L_BT))
    # ---- encC ----
    C10 = center(NUM_CLASSES)
    Wq, Wk, Wv = W['CWin'][0][:10], W['CWin'][0][10:20], W['CWin'][0][20:]
    pk.add("Cq0", bd(Wq.T, C_BT))
    pk.add("Ck0", bd(Wk.T, C_BT))
    pk.add("Cv0", bd(Wv.T, C_BT))
    pk.add("Cwo0", bd((C10 @ W['CWout'][0]).T, C_BT))
    pk.add("Cf10", bd(W['CWf1'][0].T, C_BT))
    pk.add("Cf20", bd((C10 @ W['CWf2'][0]).T, C_BT))
    pk.add("CC", bd(C10, C_BT))
    pk.add("Cones", bd(np.ones((10, 1), np.float32), C_BT))
    pk.add("Cbc", bd(np.ones((1, 10), np.float32), C_BT))
    pk.add("wdrep", np.tile(W['Wdec'][0][None, :], (120, 1)))
    # ---- vecs: per-partition LN scale columns ----
    NV = 27
    vecs = np.zeros((128, NV), np.float32)
    vecs[:, 26] = EPS
    for l in range(6):
        vecs[:121, 2 * l] = np.tile(W['pln1'][l], P_BT)
        vecs[:121, 2 * l + 1] = np.tile(W['pln2'][l], P_BT)
        vecs[:98, 12 + 2 * l] = np.tile(W['Lln1'][l], L_BT)
        vecs[:98, 12 + 2 * l + 1] = np.tile(W['Lln2'][l], L_BT)
    vecs[:120, 24] = np.tile(W['Cln1'][0], C_BT)
    vecs[:120, 25] = np.tile(W['Cln2'][0], C_BT)
    return pk, vecs


# --------------------------------------------------------------------------
# device kernel
# --------------------------------------------------------------------------

def _patch_tail_drain(tile_mod, ScopedClock, VectorClock):
    """This walrus build can't encode one drain carrying many sem waits;
    split the TileContext tail drain into one single-wait drain per proc."""
    if getattr(tile_mod.TileContext, "_tail_patched", False):
        return

    def _drain_and_barrier(self, tick_clock, wait_clock):
        gc = tick_clock.global_clock
        n = len(gc)
        for i in range(n):
            t = gc[i]
            if t <= 0:
                continue
            vec = [0] * n
            vec[i] = t
            d = self.nc.sync.drain()
            wait_clock.add_sem_waits(d.ins, ScopedClock({None: VectorClock(vec)}))
        self.nc.sync.drain()
        self.nc.all_engine_barrier()
        assert self.sems is not None
        popped = self.nc._tile_sem_poison_stack.pop()
        assert popped is self._sem_poison
        self.nc.clear_and_free_semaphores(list(self.sems.allocated().values()))
        self.nc.all_engine_barrier()

    tile_mod.TileContext._drain_and_barrier = _drain_and_barrier
    tile_mod.TileContext._tail_patched = True


def build_bass_program():
    import concourse.bass as bass
    import concourse.mybir as mybir
    import concourse.tile as tile_mod
    import concourse.tile_sem_assignment as tsa
    from concourse.vector_clock import ScopedClock, VectorClock

    _patch_tail_drain(tile_mod, ScopedClock, VectorClock)
    # this walrus rejects instructions carrying many sem waits; keep every
    # HWDGE DMA on one proc/sem (all DMAs issue on the SP ring -> FIFO-safe)
    tsa.NUM_HWDGE_SEMS = 1

    f32 = mybir.dt.float32
    AF = mybir.ActivationFunctionType
    ALU = mybir.AluOpType
    AX = mybir.AxisListType

    pk, vecs_arr = _PACKS  # layout only (values already in _PACKS arrays)
    NW = pk.n

    nc = bass.Bass("TRN2", target_bir_lowering=False, debug=False, num_devices=1)
    x0_d = nc.dram_tensor("x0", [121, F_P], f32, kind="ExternalInput")
    wp_d = nc.dram_tensor("wp", [128, NW], f32, kind="ExternalInput")
    vec_d = nc.dram_tensor("vecs", [128, 27], f32, kind="ExternalInput")
    y_d = nc.dram_tensor("y", [120, C_B2], f32, kind="ExternalOutput")
    zp_d = nc.dram_tensor("zp_scr", [121 * F_P], f32, kind="Internal")
    zl_d = nc.dram_tensor("zl_scr", [98 * F_L], f32, kind="Internal")

    def APX(t, free_dims, extra_off=0):
        # t: a 2D [parts, F] tile AP; free_dims: [[step, count], ...] within a row
        pstep, pcnt = t.ap[0]
        return bass.AP(tensor=t.tensor, offset=t.offset + extra_off,
                       ap=[[pstep, pcnt]] + free_dims)

    with tile_mod.TileContext(nc) as tc:
        ctx_pools = {}

        def wap(name):
            c0, K, M = pk.off[name]
            return wtile[0:K, c0:c0 + M]

        with tc.tile_pool(name="persist", bufs=1) as persist:
            wtile = persist.tile([128, NW], f32)
            nc.sync.dma_start(wtile[:], wp_d[:])
            vtile = persist.tile([128, 27], f32)
            nc.sync.dma_start(vtile[:], vec_d[:])

            # ---------------- generic building blocks ----------------
            def mm_to_sbuf(psum, terms, out_sb, parts, chunks, func=AF.Copy,
                           tag="mm"):
                """psum-accumulated matmul terms, then ACT func -> out_sb."""
                for (c0, cn) in chunks:
                    ps = psum.tile([parts, cn], f32, tag=tag)
                    n = len(terms)
                    for i, (lhsT, rhs) in enumerate(terms):
                        nc.tensor.matmul(ps[:, :], lhsT, rhs[:, c0:c0 + cn],
                                         start=(i == 0), stop=(i == n - 1))
                    if func == AF.Copy:
                        nc.scalar.copy(out_sb[:, c0:c0 + cn], ps[:, :])
                    else:
                        nc.scalar.activation(out_sb[:, c0:c0 + cn], ps[:, :], func)

            def layer_norm(psum, sb, terms, parts, bt, d, F, chunks, w_ap,
                           Cm, ones, bc, out_sb):
                xc = sb.tile([parts, F], f32, tag="xc")
                sq = sb.tile([parts, F], f32, tag="sq")
                sd = sb.tile([bt, F], f32, tag="sd")
                for (c0, cn) in chunks:
                    ps = psum.tile([parts, cn], f32, tag="mm")
                    allt = [(Cm, terms[0][1])] + [(m, r) for (m, r) in terms[1:]]
                    n = len(allt)
                    for i, (lhsT, rhs) in enumerate(allt):
                        nc.tensor.matmul(ps[:, :], lhsT, rhs[:, c0:c0 + cn],
                                         start=(i == 0), stop=(i == n - 1))
                    nc.scalar.copy(xc[:, c0:c0 + cn], ps[:, :])
                    nc.scalar.activation(sq[:, c0:c0 + cn], xc[:, c0:c0 + cn],
                                         AF.Square)
                    vps = psum.tile([bt, cn], f32, tag="mmv")
                    nc.tensor.matmul(vps[:, :], ones, sq[:, c0:c0 + cn])
                    nc.scalar.activation(sd[:, c0:c0 + cn], vps[:, :], AF.Ln,
                                         bias=vtile[0:bt, 26:27], scale=1.0 / d)
                nc.scalar.activation(sd[:, :], sd[:, :], AF.Exp, scale=-0.5)
                for (c0, cn) in chunks:
                    bps = psum.tile([parts, cn], f32, tag="mm")
                    nc.tensor.matmul(bps[:, :], bc, sd[:, c0:c0 + cn])
                    nc.vector.scalar_tensor_tensor(
                        out=out_sb[:, c0:c0 + cn], in0=xc[:, c0:c0 + cn],
                        scalar=w_ap, in1=bps[:, :],
                        op0=ALU.mult, op1=ALU.mult)

            def attention(psum, sb, spool, x, lq, lk, lv, parts, b2, ntok, F,
                          chunks, o_out):
                q = sb.tile([parts, F], f32, tag="q")
                k = sb.tile([parts, F], f32, tag="k")
                v = sb.tile([parts, F], f32, tag="v")
                mm_to_sbuf(psum, [(lq, x)], q, parts, chunks)
                mm_to_sbuf(psum, [(lk, x)], k, parts, chunks)
                mm_to_sbuf(psum, [(lv, x)], v, parts, chunks)
                den = sb.tile([parts, F], f32, tag="den")
                num = sb.tile([parts, F], f32, tag="num")
                # chunk over b2 so the score tile stays bounded
                bstep = max(1, min(b2, 6000 // (ntok * ntok)))
                for b0 in range(0, b2, bstep):
                    bn = min(bstep, b2 - b0)
                    s = spool.tile([parts, bstep * ntok * ntok], f32, tag="s")
                    q4 = APX(q, [[ntok, bn], [1, ntok], [0, ntok]], b0 * ntok)
                    k4 = APX(k, [[ntok, bn], [0, ntok], [1, ntok]], b0 * ntok)
                    v4 = APX(v, [[ntok, bn], [0, ntok], [1, ntok]], b0 * ntok)
                    s4 = APX(s, [[ntok * ntok, bn], [ntok, ntok], [1, ntok]])
                    s3 = APX(s, [[ntok, bn * ntok], [1, ntok]])
                    nc.vector.tensor_mul(s4, q4, k4)
                    nc.scalar.activation(s[:, :], s[:, :], AF.Exp)
                    nc.vector.tensor_reduce(
                        den[:, b0 * ntok:(b0 + bn) * ntok], s3,
                        axis=AX.X, op=ALU.add)
                    nc.vector.tensor_mul(s4, s4, v4)
                    nc.vector.tensor_reduce(
                        num[:, b0 * ntok:(b0 + bn) * ntok], s3,
                        axis=AX.X, op=ALU.add)
                nc.scalar.activation(den[:, :], den[:, :], AF.Ln)
                nc.scalar.activation(den[:, :], den[:, :], AF.Exp, scale=-1.0)
                nc.vector.tensor_mul(o_out[:, :], num[:, :], den[:, :])

            def enc_layer(psum, sb, spool, x_sb, pre, l, parts, bt, d, b2,
                          ntok, F, chunks, hid_terms, w1_ap, w2_ap, Cm, ones,
                          bc):
                o = sb.tile([parts, F], f32, tag="o")
                attention(psum, sb, spool, x_sb, wap(f"{pre}q{l}"),
                          wap(f"{pre}k{l}"), wap(f"{pre}v{l}"), parts, b2,
                          ntok, F, chunks, o)
                x1 = sb.tile([parts, F], f32, tag="x1")
                layer_norm(psum, sb, [(None, x_sb), (wap(f"{pre}wo{l}"), o)],
                           parts, bt, d, F, chunks, w1_ap, Cm, ones, bc, x1)
                # FFN
                f2_terms = []
                for (f1name, f2name, hparts) in hid_terms(l):
                    h = sb.tile([hparts, F], f32, tag=f"h{f1name[-1]}")
                    mm_to_sbuf(psum, [(wap(f1name), x1)], h, hparts, chunks,
                               func=AF.Relu)
                    f2_terms.append((wap(f2name), h))
                x2 = sb.tile([parts, F], f32, tag="x2")
                layer_norm(psum, sb, [(None, x1)] + f2_terms,
                           parts, bt, d, F, chunks, w2_ap, Cm, ones, bc, x2)
                return x2

            # ---------------- stage A: encP ----------------
            with tc.tile_pool(name="sbP", bufs=1) as sbP, \
                 tc.tile_pool(name="ssP", bufs=2) as ssP, \
                 tc.tile_pool(name="psP", bufs=2, space="PSUM") as psP:
                x = sbP.tile([121, F_P], f32, tag="x0")
                nc.sync.dma_start(x[:], x0_d[:])
                x0_keep = x

                def hidP(l):
                    return [(f"Pf1{l}_0", f"Pf2{l}_0", 128),
                            (f"Pf1{l}_1", f"Pf2{l}_1", 128),
                            (f"Pf1{l}_2", f"Pf2{l}_2", 96)]

                for l in range(6):
                    x = enc_layer(psP, sbP, ssP, x, "P", l, 121, P_BT, D_PAD,
                                  P_B2, 9, F_P, CHUNKS_P, hidP,
                                  vtile[0:121, 2 * l:2 * l + 1],
                                  vtile[0:121, 2 * l + 1:2 * l + 2],
                                  wap("PC"), wap("Pones"), wap("Pbc"))

                # cp + z build
                eh = sbP.tile([121, F_P], f32, tag="eh")
                nc.scalar.activation(eh[:, :], x[:, :], AF.Exp)
                dps = psP.tile([11, F_P], f32, tag="mmv")
                nc.tensor.matmul(dps[:, :], wap("PselS"), eh[:, :])
                mps = psP.tile([11, F_P], f32, tag="mmv2")
                nc.tensor.matmul(mps[:, :], wap("PselP"), x0_keep[:, :])
                denr = sbP.tile([11, F_P], f32, tag="denr")
                nc.scalar.activation(denr[:, :], dps[:, :], AF.Ln)
                nc.scalar.activation(denr[:, :], denr[:, :], AF.Exp, scale=-1.0)
                scl = sbP.tile([11, F_P], f32, tag="scl")
                nc.vector.tensor_mul(scl[:, :], denr[:, :], mps[:, :])
                sps = psP.tile([121, F_P], f32, tag="mm")
                nc.tensor.matmul(sps[:, :], wap("Pbc"), scl[:, :])
                zp = sbP.tile([121, F_P], f32, tag="zp")
                nc.vector.tensor_mul(zp[:, :], eh[:, :], sps[:, :])
                nc.vector.tensor_add(zp[:, :], zp[:, :], x0_keep[:, :])
                for bt in range(P_BT):
                    src = zp[bt * 11:(bt + 1) * 11, :].rearrange(
                        "c (b t) -> c b t", t=9)
                    dst = bass.AP(tensor=zp_d, offset=bt * 21 * 99,
                                  ap=[[1, 11], [99, 21], [11, 9]])
                    nc.sync.dma_start(dst, src)

            # ---------------- A -> B remap ----------------
            with tc.tile_pool(name="sbL", bufs=1) as sbL, \
                 tc.tile_pool(name="ssL", bufs=2) as ssL, \
                 tc.tile_pool(name="psL", bufs=2, space="PSUM") as psL:
                zl = sbL.tile([98, F_L], f32, tag="x")
                nc.gpsimd.memset(zl[:, :], 0.0)
                for btL in range(L_BT):
                    for di in range(3):
                        dst = zl[btL * 49 + 7 * di:btL * 49 + 7 * di + 3, :
                                 ].rearrange("p (b c) -> p b c", c=10)
                        src = bass.AP(
                            tensor=zp_d,
                            offset=btL * L_B2 * 99 + 33 * di,
                            ap=[[11, 3], [99, L_B2], [1, 10]])
                        nc.sync.dma_start(dst, src)

                # ---------------- stage B: encL ----------------
                def hidL(l):
                    return [(f"Lf1{l}", f"Lf2{l}", 2)]

                xL = zl
                for l in range(6):
                    xL = enc_layer(psL, sbL, ssL, xL, "L", l, 98, L_BT, L,
                                   L_B2, 10, F_L, CHUNKS_L, hidL,
                                   vtile[0:98, 12 + 2 * l:12 + 2 * l + 1],
                                   vtile[0:98, 12 + 2 * l + 1:12 + 2 * l + 2],
                                   wap("LC"), wap("Lones"), wap("Lbc"))
                for btL in range(L_BT):
                    src = xL[btL * 49:(btL + 1) * 49, :].rearrange(
                        "l (b c) -> l b c", c=10)
                    dst = bass.AP(tensor=zl_d, offset=btL * L_B2 * 490,
                                  ap=[[1, 49], [490, L_B2], [49, 10]])
                    nc.sync.dma_start(dst, src)

            # ---------------- B -> C remap + stage C ----------------
            with tc.tile_pool(name="sbC", bufs=1) as sbC, \
                 tc.tile_pool(name="psC", bufs=2, space="PSUM") as psC, \
                 tc.tile_pool(name="ssC", bufs=2) as ssC:
                zc = sbC.tile([120, F_C], f32, tag="x")
                for btC in range(C_BT):
                    src = bass.AP(tensor=zl_d, offset=C_B2 * btC * 490,
                                  ap=[[49, 10], [490, C_B2], [1, 49]])
                    dst = zc[btC * 10:btC * 10 + 10, :].rearrange(
                        "p (b l) -> p b l", l=49)
                    nc.sync.dma_start(dst, src)

                def hidC(l):
                    return [("Cf10", "Cf20", 12)]

                xC = enc_layer(psC, sbC, ssC, zc, "C", 0, 120, C_BT,
                               NUM_CLASSES, C_B2, L, F_C, CHUNKS_C, hidC,
                               vtile[0:120, 24:25], vtile[0:120, 25:26],
                               wap("CC"), wap("Cones"), wap("Cbc"))

                # decode: y = sum_l xC * Wdec[l]
                wd = wap("wdrep")
                tprod = sbC.tile([120, F_C], f32, tag="tp")
                wd3 = APX(wd, [[0, C_B2], [1, 49]])
                x3 = APX(xC, [[49, C_B2], [1, 49]])
                t3 = APX(tprod, [[49, C_B2], [1, 49]])
                nc.vector.tensor_mul(t3, x3, wd3)
                ytile = sbC.tile([120, C_B2], f32, tag="y")
                nc.vector.tensor_reduce(
                    ytile[:, :], APX(tprod, [[49, C_B2], [1, 49]]),
                    axis=AX.X, op=ALU.add)
                nc.sync.dma_start(y_d[:], ytile[:, :])

    # walrus in this toolchain enforces <=1 sem wait per instruction
    # (2 for EventSemaphore); run the bacc normalization passes.
    import bass_rust as _bass_rust
    _bass_rust.move_matmul_waits_to_ldweights(nc.m)
    _bass_rust.generate_event_semaphores(nc)
    return nc


def APX_D(bass, dram_handle, off, ap):
    return bass.AP(tensor=dram_handle, offset=off, ap=ap)


_PACKS = None


def _install_ntff_hook():
    """This image's antenv lacks axon_hooks; synthesize it so trace=True
    can capture NTFF profiles via the injected libaxon_pjrt.so."""
    import types
    try:
        import antenv.axon_hooks  # noqa: F401
        return
    except ImportError:
        pass
    try:
        from trn_agent_boot.trn_boot import _ntff_profile_via_ctypes
    except ImportError:
        sys.path.insert(0, os.path.expanduser("~/.axon_site"))
        from trn_agent_boot.trn_boot import _ntff_profile_via_ctypes
    hook = None
    for so in ("/opt/axon/libaxon_pjrt.so",):
        if os.path.exists(so):
            hook = _ntff_profile_via_ctypes(so)
            break
    mod = types.ModuleType("antenv.axon_hooks")
    mod.get_axon_ntff_profile_hook = lambda: hook
    mod.set_axon_ntff_profile_hook = lambda h: None
    import antenv
    antenv.axon_hooks = mod
    sys.modules["antenv.axon_hooks"] = mod


def kernel(**inputs):
    global _PACKS
    W = {k: np.asarray(v, np.float32) for k, v in inputs.items()}
    x_full = W.pop('x')
    pk, vecs_arr = build_packs(W)
    _PACKS = (pk, vecs_arr)
    wpack_arr = pk.array()

    nc = build_bass_program()

    from concourse.bass_utils import run_bass_kernel_spmd
    trace = os.environ.get("KERNEL_TRACE", "") == "1"
    if trace:
        _install_ntff_hook()
    in_maps = []
    for core in range(8):
        in_maps.append({
            "x0": build_x0(x_full, core),
            "wp": wpack_arr,
            "vecs": vecs_arr,
        })
    res = run_bass_kernel_spmd(nc, in_maps, core_ids=list(range(8)),
                               trace=trace)
    kernel.last_result = res
    ys = [res.results[i]["y"] for i in range(8)]
    return assemble_output(ys)


if __name__ == "__main__":
    rng = np.random.default_rng(0)
    print("building program only (syntax check)...")
    # minimal fake weights for a build check
    W = {
        'pWin': rng.standard_normal((6, 33, 11)), 'pWout': rng.standard_normal((6, 11, 11)),
        'pWf1': rng.standard_normal((6, 32, 11)), 'pWf2': rng.standard_normal((6, 11, 32)),
        'pln1': np.ones((6, 11)), 'pln2': np.ones((6, 11)),
        'LWin': rng.standard_normal((6, 147, 49)), 'LWout': rng.standard_normal((6, 49, 49)),
        'LWf1': rng.standard_normal((6, 1, 49)), 'LWf2': rng.standard_normal((6, 49, 1)),
        'Lln1': np.ones((6, 49)), 'Lln2': np.ones((6, 49)),
        'CWin': rng.standard_normal((1, 30, 10)), 'CWout': rng.standard_normal((1, 10, 10)),
        'CWf1': rng.standard_normal((1, 1, 10)), 'CWf2': rng.standard_normal((1, 10, 1)),
        'Cln1': np.ones((1, 10)), 'Cln2': np.ones((1, 10)),
        'Wdec': rng.standard_normal((1, 49)),
    }
    W = {k: np.asarray(v, np.float32) for k, v in W.items()}
    pk, vecs_arr = build_packs(W)
    _PACKS = (pk, vecs_arr)
    print("wpack cols:", pk.n)
    nc = build_bass_program()
    print("program built OK")
